# revision 1
# baseline (speedup 1.0000x reference)
"""Fused self-attention kernel for Trainium2 (Bass/Tile), SPMD over 8 cores.

Math (per batch b):
    q = x @ Wq + bq ; k = x @ Wk + bk ; v = x @ Wv + bv          [T, C]
    scores[t, s] = k[t] . q[s]      (non-causal, unscaled)
    beta = softmax(scores, axis=s)
    attn[t] = sum_s beta[t, s] * v[s]
    out = gamma * attn + x

Sharding: 8 cores = 4 batches x 2 halves of the output rows t. Each core
receives its batch's x rotated so its local 2048 output rows come first
(softmax/attention over s is permutation invariant, so rotating s is safe).
All cores run the identical program on different data.

On-chip layout: scoresT[s, t] = qT.T @ kT is computed with s on partitions
and t on the free axis; the softmax denominator comes for free by appending
a ones column to V (attn_aug = [V | 1].T @ exp(scoresT)).  No max-subtraction
is needed: |scores| < ~60 for any remotely normalized input, and exp is
evaluated in fp32 (overflow threshold 88).  The T x T score matrix never
touches HBM.
"""

import numpy as np
from contextlib import ExitStack

import concourse.bass as bass
import concourse.tile as tile
from concourse import bacc, mybir
from concourse.bass_utils import run_bass_kernel_spmd
from concourse.masks import make_identity

FP32 = mybir.dt.float32
BF16 = mybir.dt.bfloat16
AF = mybir.ActivationFunctionType

B, T, C = 4, 4096, 64
CA = C + 1            # x gets a ones column appended (folds biases into matmuls)
HALVES = 2            # cores per batch
N_CORES = B * HALVES
T_LOC = T // HALVES   # output rows per core
P = 128
NT = T // P           # 32 s-tiles of 128
TB = 1024             # t-block width (two PSUM banks; bf16 moving max)
N_TB = T_LOC // TB    # 2
SB = 512              # qT column chunk width
NT_MAIN = NT          # s-tiles processed in the main loop (debug knob)


def _emit(tc, ctx, x_d, wq_d, wk_d, wv_d, bq_d, bk_d, bv_d, g_d, out_d):
    nc = tc.nc

    const = ctx.enter_context(tc.tile_pool(name="const", bufs=1))
    setup = ctx.enter_context(tc.tile_pool(name="setup", bufs=2))
    expp = ctx.enter_context(tc.tile_pool(name="expp", bufs=6))
    osbp = ctx.enter_context(tc.tile_pool(name="osbp", bufs=2))
    outp = ctx.enter_context(tc.tile_pool(name="outp", bufs=6))
    smallp = ctx.enter_context(tc.tile_pool(name="smallp", bufs=8))
    # PSUM budget (8 banks): scores [128,1024] x2 bufs = 4, the two
    # persistent attn accumulators [65,1024] = 4.  The finalize-phase
    # transpose tiles share the scores tag (scores allocation has stopped
    # by then).
    ps_big = ctx.enter_context(tc.tile_pool(name="ps_big", bufs=2, space="PSUM"))
    ps_o = ctx.enter_context(tc.tile_pool(name="ps_o", bufs=1, space="PSUM"))

    # ---- constants ------------------------------------------------------
    ident = const.tile([P, P], FP32, tag="ident")
    make_identity(nc, ident)

    g128 = const.tile([P, 1], FP32, tag="g128")
    nc.sync.dma_start(g128, g_d.ap().to_broadcast([P, 1]))

    def w_aug(w_d, b_d, name):
        # [128, 128] bf16, zero padded: rows 0:C = W, row C = bias (the ones
        # column of x_aug multiplies it back in), rest zero.  Full-K/M shapes
        # keep the PE HAM activity monitor seeing full-array matmuls (K<128
        # matmuls never un-throttle the 1.2->2.4 GHz clock gate).
        w = const.tile([P, P], BF16, tag=name)
        nc.vector.memset(w, 0.0)
        tw = setup.tile([C, C], FP32, tag="tw")
        nc.sync.dma_start(tw, w_d.ap())
        nc.vector.tensor_copy(w[0:C, 0:C], tw)
        tb_ = setup.tile([1, C], FP32, tag="tb")
        nc.sync.dma_start(tb_, b_d.ap()[None, :])
        nc.vector.tensor_copy(w[C:CA, 0:C], tb_)
        return w

    wq = w_aug(wq_d, bq_d, "wq")
    wk = w_aug(wk_d, bk_d, "wk")
    wv = w_aug(wv_d, bv_d, "wv")

    # ---- load x, build xT ----------------------------------------------
    x_v = x_d.ap().rearrange("(n p) c -> p n c", p=P)  # [128, 32, 65]
    x_nat = const.tile([P, NT, CA], FP32, tag="xnat")
    for i in range(8):
        nc.sync.dma_start(x_nat[:, i * 4:(i + 1) * 4, :], x_v[:, i * 4:(i + 1) * 4, :])

    xT = const.tile([P, T], BF16, tag="xT")  # rows: 0:C x.T, C ones, rest 0
    # zero the pad rows (64:128); the ones row (64) is rewritten by the
    # transpose copies below.  gpsimd wants 32-aligned start partitions.
    nc.gpsimd.memset(xT[C:P, :], 0.0)
    for g in range(T // TB):
        psx = ps_big.tile([P, TB], FP32, tag="big")
        for j in range(TB // P):
            idx = g * (TB // P) + j
            nc.tensor.transpose(psx[0:CA, j * P:(j + 1) * P], x_nat[:, idx, :], ident)
        nc.vector.tensor_copy(xT[0:CA, g * TB:(g + 1) * TB], psx[0:CA, :])

    # ---- projections ----------------------------------------------------
    # qT[d, s] over all s; kT[d, t] over local t; v_aug[s, C+1] over all s.
    qt = []
    for i in range(T // SB):
        ps = ps_big.tile([P, SB], FP32, tag="big")
        nc.tensor.matmul(ps, lhsT=wq, rhs=xT[:, i * SB:(i + 1) * SB],
                         start=True, stop=True)
        q_sb = const.tile([P, SB], BF16, tag=f"qt{i}")
        if i % 2 == 0:
            nc.vector.tensor_copy(q_sb, ps)
        else:
            nc.scalar.copy(q_sb, ps)
        qt.append(q_sb)

    kt = []
    for i in range(T_LOC // TB):
        k_sb = const.tile([P, TB], BF16, tag=f"kt{i}")
        for j in range(TB // SB):
            ps = ps_big.tile([P, SB], FP32, tag="big")
            nc.tensor.matmul(ps, lhsT=wk,
                             rhs=xT[:, i * TB + j * SB:i * TB + (j + 1) * SB],
                             start=True, stop=True)
            if j % 2 == 0:
                nc.vector.tensor_copy(k_sb[:, j * SB:(j + 1) * SB], ps)
            else:
                nc.scalar.copy(k_sb[:, j * SB:(j + 1) * SB], ps)
        kt.append(k_sb)

    va = []
    for g in range(NT // 8):
        ps = ps_big.tile([P, 8 * C], FP32, tag="big")
        for j in range(8):
            idx = g * 8 + j
            nc.tensor.matmul(ps[:, j * C:(j + 1) * C],
                             lhsT=xT[:, idx * P:(idx + 1) * P], rhs=wv[:, 0:C],
                             start=True, stop=True)
        v_sb = const.tile([P, 8, P], BF16, tag=f"va{g}")
        nc.vector.tensor_copy(v_sb[:, :, 0:C], ps.rearrange("p (n c) -> p n c", c=C))
        nc.vector.memset(v_sb[:, :, C:CA], 1.0)
        nc.vector.memset(v_sb[:, :, CA:P], 0.0)
        va.append(v_sb)

    # ---- flash attention main loop --------------------------------------
    # s-tile outer loop: per s-tile load qt/va stationary weights once and
    # stream both 1024-wide t-blocks; both attn accumulators are persistent
    # in PSUM.  Software-pipelined: scores for s-tile st+1 are emitted before
    # the attn matmuls of s-tile st so PE never waits on ACT's exp.
    out_v = out_d.ap().rearrange("(n p) c -> p n c", p=P)  # [128, 16, 64]

    po = [ps_o.tile([P, TB], FP32, tag=f"o{tb}", name="po") for tb in range(N_TB)]
    ex = [None] * NT

    def scores(tb, st):
        pss = ps_big.tile([P, TB], FP32, tag="big", name="pss")
        for h in range(TB // SB):
            nc.tensor.matmul(
                pss[:, h * SB:(h + 1) * SB],
                lhsT=qt[st // 4][:, (st % 4) * P:(st % 4 + 1) * P],
                rhs=kt[tb][:, h * SB:(h + 1) * SB], start=True, stop=True)
        e = expp.tile([P, TB], BF16, tag="ex", name="ex")
        nc.scalar.activation(e, pss, AF.Exp)
        ex[st] = e

    def attn(tb, st):
        for h in range(TB // SB):  # matmul dst must stay in one PSUM bank
            nc.tensor.matmul(po[tb][:, h * SB:(h + 1) * SB],
                             lhsT=va[st // 8][:, st % 8, :],
                             rhs=ex[st][:, h * SB:(h + 1) * SB],
                             start=(st == 0), stop=(st == NT_MAIN - 1))

    def finalize(tb):
        # transpose [128, 128] chunks back, normalize, gamma, residual, store
        osb = osbp.tile([P, TB], FP32, tag="osb")
        nc.vector.tensor_copy(osb, po[tb])
        for j in range(TB // P):
            # alternate psum slots: po[tb]'s slot is free once osb is copied
            if j % 2 == 0:
                pt = ps_big.tile([P, P], FP32, tag="big", name="pt")
            else:
                pt = ps_o.tile([P, P], FP32, tag=f"o{tb}", name="pt")
            nc.tensor.transpose(pt, osb[:, j * P:(j + 1) * P], ident)
            rec = smallp.tile([P, 1], FP32, tag="rec")
            nc.vector.reciprocal(rec, pt[:, C:CA])
            grec = smallp.tile([P, 1], FP32, tag="grec")
            nc.vector.tensor_mul(grec, rec, g128)
            ot = outp.tile([P, C], FP32, tag="ot")
            nc.vector.tensor_scalar_mul(ot, pt[:, 0:C], grec)
            idx = tb * (TB // P) + j
            nc.vector.tensor_add(ot, ot, x_nat[:, idx, 0:C])
            nc.sync.dma_start(out_v[:, idx, :], ot)

    # two sequential phases (one per t-block): tb=0's finalize overlaps
    # tb=1's compute on otherwise-idle engines.
    for tb in range(N_TB):
        scores(tb, 0)
        for st in range(1, NT_MAIN):
            scores(tb, st)
            attn(tb, st - 1)
        attn(tb, NT_MAIN - 1)
        finalize(tb)


def build():
    nc = bacc.Bacc("TRN2", target_bir_lowering=False, debug=False,
                   num_devices=N_CORES)
    x_d = nc.dram_tensor("x", [T, CA], FP32, kind="ExternalInput")
    wq_d = nc.dram_tensor("wq", [C, C], FP32, kind="ExternalInput")
    wk_d = nc.dram_tensor("wk", [C, C], FP32, kind="ExternalInput")
    wv_d = nc.dram_tensor("wv", [C, C], FP32, kind="ExternalInput")
    bq_d = nc.dram_tensor("bq", [C], FP32, kind="ExternalInput")
    bk_d = nc.dram_tensor("bk", [C], FP32, kind="ExternalInput")
    bv_d = nc.dram_tensor("bv", [C], FP32, kind="ExternalInput")
    g_d = nc.dram_tensor("gamma", [1], FP32, kind="ExternalInput")
    out_d = nc.dram_tensor("out", [T_LOC, C], FP32, kind="ExternalOutput")

    with tile.TileContext(nc) as tc, ExitStack() as ctx:
        _emit(tc, ctx, x_d, wq_d, wk_d, wv_d, bq_d, bk_d, bv_d, g_d, out_d)
    nc.compile()
    return nc


def make_in_maps(inputs, Wq, bq, Wk, bk, Wv, bv, gamma):
    """Shard the full inputs into per-core input maps."""
    x = np.asarray(inputs, dtype=np.float32).reshape(B, T, C)
    ones = np.ones((T, 1), dtype=np.float32)
    in_maps = []
    for core in range(N_CORES):
        b, h = divmod(core, HALVES)
        xb = x[b]
        if h:
            xb = np.concatenate([xb[h * T_LOC:], xb[:h * T_LOC]], axis=0)
        x_aug = np.ascontiguousarray(np.concatenate([xb, ones], axis=1))
        in_maps.append({
            "x": x_aug,
            "wq": np.asarray(Wq, np.float32), "bq": np.asarray(bq, np.float32),
            "wk": np.asarray(Wk, np.float32), "bk": np.asarray(bk, np.float32),
            "wv": np.asarray(Wv, np.float32), "bv": np.asarray(bv, np.float32),
            "gamma": np.asarray(gamma, np.float32),
        })
    return in_maps


def assemble(results):
    """Gather per-core [T_LOC, C] outputs into the full [B, 1, T, C]."""
    out = np.empty((B, 1, T, C), dtype=np.float32)
    for core in range(N_CORES):
        b, h = divmod(core, HALVES)
        out[b, 0, h * T_LOC:(h + 1) * T_LOC, :] = results[core]["out"]
    return out


_NC_CACHE = []


def kernel(inputs, Wq, bq, Wk, bk, Wv, bv, gamma):
    if not _NC_CACHE:
        _NC_CACHE.append(build())
    nc = _NC_CACHE[0]
    in_maps = make_in_maps(inputs, Wq, bq, Wk, bk, Wv, bv, gamma)
    res = run_bass_kernel_spmd(nc, in_maps, list(range(N_CORES)))
    return assemble(res.results)



# revision 2
# speedup vs baseline: 1.1815x; 1.1815x over previous
"""Fused self-attention kernel for Trainium2 (Bass/Tile), SPMD over 8 cores.

Math (per batch b):
    q = x @ Wq + bq ; k = x @ Wk + bk ; v = x @ Wv + bv          [T, C]
    scores[t, s] = k[t] . q[s]      (non-causal, unscaled)
    beta = softmax(scores, axis=s)
    attn[t] = sum_s beta[t, s] * v[s]
    out = gamma * attn + x

Sharding: 8 cores = 4 batches x 2 halves of the output rows t. Each core's
x is rotated so its local 2048 output rows come first (softmax over s is
permutation invariant, so rotating s is safe).

Host-side layout prep (inside kernel(), pure numpy): x is transposed,
ones-augmented and cast to bf16 (xt), the residual slice is pre-tiled
to the SBUF partition layout (xr), and the weights are padded to
[128,128] bf16 with the bias folded in as row 64 (the ones row of xt
multiplies it back in).  gamma is folded into Wv/bv (the ones column of
v_aug that produces the softmax denominator is NOT scaled), so the
device kernel never sees gamma and out = num/denom + x directly.

On-chip: scoresT[s, t] with s on partitions; denominator via the ones
column of v_aug.  No max-subtraction (|scores| < ~60 for normalized
inputs; exp in fp32, overflow at 88).  exp is split between the Scalar
engine (table exp) and the Vector engine (Schraudolph-style bf16 bit
trick: bitcast(round(x*184.665 + 16248.7)) ~= exp(x) to ~3%), since ACT
alone (1 elem/cycle/lane) would be the bottleneck.  The TxT score
matrix never touches HBM.
"""

import numpy as np
import ml_dtypes
from contextlib import ExitStack

import concourse.bass as bass
import concourse.tile as tile
from concourse import bacc, mybir
from concourse.bass_utils import run_bass_kernel_spmd
from concourse.masks import make_identity

FP32 = mybir.dt.float32
BF16 = mybir.dt.bfloat16
I16 = mybir.dt.int16
AF = mybir.ActivationFunctionType
ALU = mybir.AluOpType

B, T, C = 4, 4096, 64
P = 128
HALVES = 2
N_CORES = B * HALVES
TL = T // HALVES      # local output rows per core (2048)
TB = 1024             # t-block width (one PSUM accumulator pair)
N_TB = TL // TB       # 2
SB = 512              # psum-bank-sized matmul free dim
NT = T // P           # 32 s-tiles

# Schraudolph exp in bf16 bits: exp(x) ~= bitcast_bf16(round(A*x + BB))
SCHRAUD_A = 128.0 / np.log(2.0)          # 184.6650
SCHRAUD_B = 16256.0 - 0.0573 * 128.0     # 16248.67 (balanced max rel err ~3%)

# which s-tiles' exp goes to the Vector engine (rest on Scalar engine)
DVE_EXP = set(range(1, NT, 3))           # 10 of 32 per t-block


def _emit(tc, ctx, xt_d, xr_d, wq_d, wk_d, wv_d, out_d):
    nc = tc.nc

    const = ctx.enter_context(tc.tile_pool(name="const", bufs=1))
    expp = ctx.enter_context(tc.tile_pool(name="expp", bufs=6))
    osbp = ctx.enter_context(tc.tile_pool(name="osbp", bufs=2))
    outp = ctx.enter_context(tc.tile_pool(name="outp", bufs=6))
    smallp = ctx.enter_context(tc.tile_pool(name="smallp", bufs=8))
    # PSUM (8 banks): pss [128,1024] x2 bufs = 4 banks, po accumulators
    # o0/o1 [128,1024] = 2 banks each.  Setup rounds and the finalize
    # transposes alias the o0/o1 regions via tags while those are free.
    ps_s = ctx.enter_context(tc.tile_pool(name="ps_s", bufs=2, space="PSUM"))
    ps_o = ctx.enter_context(tc.tile_pool(name="ps_o", bufs=1, space="PSUM"))

    # ---- constants & DMAs ----------------------------------------------
    ident = const.tile([P, P], BF16, tag="ident")
    make_identity(nc, ident)

    wq = const.tile([P, P], BF16, tag="wq")
    wk = const.tile([P, P], BF16, tag="wk")
    wv = const.tile([P, P], BF16, tag="wv")
    nc.sync.dma_start(wq, wq_d.ap())
    nc.sync.dma_start(wk, wk_d.ap())
    nc.sync.dma_start(wv, wv_d.ap())

    # preload the exp activation table while DMAs run (first ACTIVATE of a
    # set pays ~2.7us; do it off the critical path)
    zt = smallp.tile([P, 1], FP32, tag="zt")
    nc.vector.memset(zt, 0.0)
    zo = smallp.tile([P, 1], FP32, tag="zo")
    nc.scalar.activation(zo, zt, AF.Exp)

    xt = const.tile([P, T], BF16, tag="xt")   # rows 0:64 x.T, 64 ones, 65: zeros
    for i in range(4):
        nc.sync.dma_start(xt[:, i * TB:(i + 1) * TB],
                          xt_d.ap()[:, i * TB:(i + 1) * TB])

    xr = const.tile([P, TL // P, C], FP32, tag="xr")  # residual, partition-tiled
    nc.sync.dma_start(xr, xr_d.ap().rearrange("p (n c) -> p n c", c=C))

    # HAM warm-up: real K=128 matmuls on junk data keep PE busy from t~0 so
    # the 1.2->2.4 GHz un-throttle window elapses during setup.
    for _ in range(8):
        dmy = ps_o.tile([P, P], FP32, tag="o1", name="dummy")
        nc.tensor.matmul(dmy, lhsT=wq, rhs=ident, start=True, stop=True)

    # ---- projections (through the o0/o1 psum regions) -------------------
    qt = const.tile([P, T], BF16, tag="qt")       # q.T, all s
    kt = const.tile([P, TL], BF16, tag="kt")      # k.T, local t
    va = const.tile([P, NT, P], BF16, tag="va")   # v_aug per s-tile [s,c]

    def kt_round(g):  # cols [g*1024, (g+1)*1024)
        ps = ps_o.tile([P, TB], FP32, tag="o1", name="ktps")
        for h in range(2):
            nc.tensor.matmul(ps[:, h * SB:(h + 1) * SB], lhsT=wk,
                             rhs=xt[:, g * TB + h * SB:g * TB + (h + 1) * SB],
                             start=True, stop=True)
        nc.scalar.copy(kt[:, g * TB:(g + 1) * TB], ps)

    def qt_round(i, tag):  # cols [i*512, (i+1)*512)
        ps = ps_o.tile([P, SB], FP32, tag=tag, name="qtps")
        nc.tensor.matmul(ps, lhsT=wq, rhs=xt[:, i * SB:(i + 1) * SB],
                         start=True, stop=True)
        nc.vector.tensor_copy(qt[:, i * SB:(i + 1) * SB], ps)

    def va_round(g, on_act):  # s-tiles [g*8, (g+1)*8)
        ps = ps_o.tile([P, 8, P], FP32, tag="o1", name="vaps")
        for j in range(8):
            nc.tensor.matmul(ps[:, j, :], lhsT=xt[:, (g * 8 + j) * P:(g * 8 + j + 1) * P],
                             rhs=wv, start=True, stop=True)
        if on_act:
            nc.scalar.copy(va[:, g * 8:(g + 1) * 8, :], ps)
        else:
            nc.vector.tensor_copy(va[:, g * 8:(g + 1) * 8, :], ps)

    # minimal upfront set: enough for (tb=0, st=0..7)
    kt_round(0)
    qt_round(0, "o0")         # o0 region is free until attn(0,0)
    va_round(0, True)

    # remaining setup rounds, interleaved into the tb=0 main loop
    setup_sched = {
        2: lambda: qt_round(1, "o1"),
        3: lambda: va_round(1, False),
        6: lambda: qt_round(2, "o1"),
        8: lambda: qt_round(3, "o1"),
        9: lambda: va_round(2, False),
        12: lambda: qt_round(4, "o1"),
        14: lambda: qt_round(5, "o1"),
        15: lambda: va_round(3, False),
        18: lambda: qt_round(6, "o1"),
        20: lambda: qt_round(7, "o1"),
        22: lambda: kt_round(1),
    }

    # ---- flash attention main loop --------------------------------------
    out_v = out_d.ap().rearrange("(n p) c -> p n c", p=P)  # [128, 16, 64]
    po = [ps_o.tile([P, TB], FP32, tag=f"o{tb}", name="po") for tb in range(N_TB)]
    ex = [None] * NT

    def scores(tb, st):
        pss = ps_s.tile([P, TB], FP32, tag="pss", name="pss")
        for h in range(2):
            nc.tensor.matmul(pss[:, h * SB:(h + 1) * SB],
                             lhsT=qt[:, st * P:(st + 1) * P],
                             rhs=kt[:, tb * TB + h * SB:tb * TB + (h + 1) * SB],
                             start=True, stop=True)
        e = expp.tile([P, TB], BF16, tag="ex", name="ex")
        if st in DVE_EXP:
            nc.vector.tensor_scalar(e.bitcast(I16), pss, SCHRAUD_A, SCHRAUD_B,
                                    ALU.mult, ALU.add)
        else:
            nc.scalar.activation(e, pss, AF.Exp)
        ex[st] = e

    def attn(tb, st):
        for h in range(2):
            nc.tensor.matmul(po[tb][:, h * SB:(h + 1) * SB],
                             lhsT=va[:, st, :],
                             rhs=ex[st][:, h * SB:(h + 1) * SB],
                             start=(st == 0), stop=(st == NT - 1))

    def finalize(tb):
        # po[c_aug, t] -> transpose 128-chunks back via identity matmuls,
        # normalize by the denominator column, add residual, store.
        osb = osbp.tile([P, TB], BF16, tag="osb")
        nc.scalar.copy(osb[:, 0:SB], po[tb][:, 0:SB])
        nc.vector.tensor_copy(osb[:, SB:TB], po[tb][:, SB:TB])
        pts = [ps_o.tile([P, 4, P], FP32, tag=f"o{tb}", name="pt") for _ in range(2)]
        for j in range(8):
            pt = pts[j // 4]
            nc.tensor.matmul(pt[:, j % 4, :], lhsT=osb[:, j * P:(j + 1) * P],
                             rhs=ident, start=True, stop=True)
            rec = smallp.tile([P, 1], FP32, tag="rec")
            nc.vector.reciprocal(rec, pt[:, j % 4, C:C + 1])
            ot = outp.tile([P, C], FP32, tag="ot")
            nc.scalar.activation(ot, pt[:, j % 4, 0:C], AF.Copy, scale=rec)
            nc.vector.tensor_add(ot, ot, xr[:, tb * 8 + j, :])
            nc.sync.dma_start(out_v[:, tb * 8 + j, :], ot)

    for tb in range(N_TB):
        for st in range(NT):
            if tb == 0 and st in setup_sched:
                setup_sched[st]()
            scores(tb, st)
            if st > 0:
                attn(tb, st - 1)
        attn(tb, NT - 1)
        finalize(tb)


def build():
    nc = bacc.Bacc("TRN2", target_bir_lowering=False, debug=False,
                   num_devices=N_CORES)
    xt_d = nc.dram_tensor("xt", [P, T], BF16, kind="ExternalInput")
    xr_d = nc.dram_tensor("xr", [P, TL // P * C], FP32, kind="ExternalInput")
    wq_d = nc.dram_tensor("wq", [P, P], BF16, kind="ExternalInput")
    wk_d = nc.dram_tensor("wk", [P, P], BF16, kind="ExternalInput")
    wv_d = nc.dram_tensor("wv", [P, P], BF16, kind="ExternalInput")
    out_d = nc.dram_tensor("out", [TL, C], FP32, kind="ExternalOutput")

    with tile.TileContext(nc) as tc, ExitStack() as ctx:
        _emit(tc, ctx, xt_d, xr_d, wq_d, wk_d, wv_d, out_d)
    nc.compile()
    return nc


def make_in_maps(inputs, Wq, bq, Wk, bk, Wv, bv, gamma):
    """Host-side layout prep + sharding into per-core input maps."""
    bf16 = ml_dtypes.bfloat16
    x = np.asarray(inputs, dtype=np.float32).reshape(B, T, C)
    g = float(np.asarray(gamma, np.float32).reshape(-1)[0])

    def w_aug(W, b, scale=1.0):
        w = np.zeros((P, P), dtype=np.float32)
        w[0:C, 0:C] = np.asarray(W, np.float32) * scale
        w[C, 0:C] = np.asarray(b, np.float32) * scale
        return w.astype(bf16)

    wq_np = w_aug(Wq, bq)
    wk_np = w_aug(Wk, bk)
    wv_np = w_aug(Wv, bv, scale=g)      # gamma folded into V
    wv_np[C, C] = bf16(1.0)             # ones column -> softmax denominator

    in_maps = []
    for core in range(N_CORES):
        b_i, h = divmod(core, HALVES)
        xb = x[b_i]
        if h:
            xb = np.concatenate([xb[h * TL:], xb[:h * TL]], axis=0)
        xt_np = np.zeros((P, T), dtype=bf16)
        xt_np[0:C] = xb.T.astype(bf16)
        xt_np[C] = bf16(1.0)
        xr_np = np.ascontiguousarray(
            xb[0:TL].reshape(TL // P, P, C).transpose(1, 0, 2).reshape(P, -1))
        in_maps.append({
            "xt": xt_np, "xr": xr_np,
            "wq": wq_np, "wk": wk_np, "wv": wv_np,
        })
    return in_maps


def assemble(results):
    """Gather per-core [TL, C] outputs into the full [B, 1, T, C]."""
    out = np.empty((B, 1, T, C), dtype=np.float32)
    for core in range(N_CORES):
        b_i, h = divmod(core, HALVES)
        out[b_i, 0, h * TL:(h + 1) * TL, :] = results[core]["out"]
    return out


_NC_CACHE = []


def kernel(inputs, Wq, bq, Wk, bk, Wv, bv, gamma):
    if not _NC_CACHE:
        _NC_CACHE.append(build())
    nc = _NC_CACHE[0]
    in_maps = make_in_maps(inputs, Wq, bq, Wk, bk, Wv, bv, gamma)
    res = run_bass_kernel_spmd(nc, in_maps, list(range(N_CORES)))
    return assemble(res.results)


# revision 6
# speedup vs baseline: 1.1834x; 1.0016x over previous
"""Fused self-attention kernel for Trainium2 (Bass/Tile), SPMD over 8 cores.

Math (per batch b):
    q = x @ Wq + bq ; k = x @ Wk + bk ; v = x @ Wv + bv          [T, C]
    scores[t, s] = k[t] . q[s]      (non-causal, unscaled)
    beta = softmax(scores, axis=s)
    attn[t] = sum_s beta[t, s] * v[s]
    out = gamma * attn + x

Sharding: 8 cores = 4 batches x 2 halves of the output rows t. Each core's
x is rotated so its local 2048 output rows come first (softmax over s is
permutation invariant, so rotating s is safe).

Host-side layout prep (inside kernel(), pure numpy): x is transposed,
ones-augmented and cast to bf16 (xt), the residual slice is pre-tiled
to the SBUF partition layout (xr), and the weights are padded to
[128,128] bf16 with the bias folded in as row 64 (the ones row of xt
multiplies it back in).  gamma is folded into Wv/bv (the ones column of
v_aug that produces the softmax denominator is NOT scaled), so the
device kernel never sees gamma and out = num/denom + x directly.

On-chip: scoresT[s, t] with s on partitions; denominator via the ones
column of v_aug.  No max-subtraction (|scores| < ~60 for normalized
inputs; exp in fp32, overflow at 88).  exp is split between the Scalar
engine (table exp) and the Vector engine (Schraudolph-style bf16 bit
trick: bitcast(round(x*184.665 + 16248.7)) ~= exp(x) to ~3%), since ACT
alone (1 elem/cycle/lane) would be the bottleneck.  The TxT score
matrix never touches HBM.
"""

import numpy as np
import ml_dtypes
from contextlib import ExitStack

import concourse.bass as bass
import concourse.tile as tile
from concourse import bacc, mybir
from concourse.bass_utils import run_bass_kernel_spmd
from concourse.masks import make_identity

FP32 = mybir.dt.float32
BF16 = mybir.dt.bfloat16
I16 = mybir.dt.int16
AF = mybir.ActivationFunctionType
ALU = mybir.AluOpType

B, T, C = 4, 4096, 64
P = 128
HALVES = 2
N_CORES = B * HALVES
TL = T // HALVES      # local output rows per core (2048)
TB = 1024             # t-block width (one PSUM accumulator pair)
N_TB = TL // TB       # 2
SB = 512              # psum-bank-sized matmul free dim
NT = T // P           # 32 s-tiles

# Schraudolph exp in bf16 bits: exp(x) ~= bitcast_bf16(round(A*x + BB))
SCHRAUD_A = 128.0 / np.log(2.0)          # 184.6650
SCHRAUD_B = 16256.0 - 0.0573 * 128.0     # 16248.67 (balanced max rel err ~3%)

# which s-tiles' exp goes to the Vector engine (rest on Scalar engine)
DVE_EXP = set(range(1, NT, 3))           # 10 of 32 per t-block


def _emit(tc, ctx, xt_d, xr_d, wq_d, wk_d, wv_d, out_d):
    nc = tc.nc

    const = ctx.enter_context(tc.tile_pool(name="const", bufs=1))
    expp = ctx.enter_context(tc.tile_pool(name="expp", bufs=6))
    osbp = ctx.enter_context(tc.tile_pool(name="osbp", bufs=2))
    outp = ctx.enter_context(tc.tile_pool(name="outp", bufs=8))
    smallp = ctx.enter_context(tc.tile_pool(name="smallp", bufs=8))
    # PSUM (8 banks): pss [128,1024] x2 bufs = 4 banks, po accumulators
    # o0/o1 [128,1024] = 2 banks each.  Setup rounds and the finalize
    # transposes alias the o0/o1 regions via tags while those are free.
    ps_s = ctx.enter_context(tc.tile_pool(name="ps_s", bufs=2, space="PSUM"))
    ps_o = ctx.enter_context(tc.tile_pool(name="ps_o", bufs=1, space="PSUM"))

    # ---- constants & DMAs ----------------------------------------------
    ident = const.tile([P, P], BF16, tag="ident")
    make_identity(nc, ident)

    wq = const.tile([P, P], BF16, tag="wq")
    wk = const.tile([P, P], BF16, tag="wk")
    wv = const.tile([P, P], BF16, tag="wv")
    xt = const.tile([P, T], BF16, tag="xt")   # rows 0:64 x.T, 64 ones, 65: zeros
    qt = const.tile([P, T], BF16, tag="qt")       # q.T, all s
    kt = const.tile([P, TL], BF16, tag="kt")      # k.T, local t
    va = const.tile([P, NT, P], BF16, tag="va")   # v_aug per s-tile [s,c]

    # DMA order: first xt chunk + weights first (setup starts on them),
    # residual last (only needed at finalize time).
    nc.sync.dma_start(xt[:, 0:TB], xt_d.ap()[:, 0:TB])
    nc.sync.dma_start(wq, wq_d.ap())
    nc.sync.dma_start(wk, wk_d.ap())
    nc.sync.dma_start(wv, wv_d.ap())
    for i in range(1, 4):
        nc.sync.dma_start(xt[:, i * TB:(i + 1) * TB],
                          xt_d.ap()[:, i * TB:(i + 1) * TB])
    xr = const.tile([P, TL // P, C], FP32, tag="xr")  # residual, partition-tiled
    nc.sync.dma_start(xr, xr_d.ap().rearrange("p (n c) -> p n c", c=C))

    # preload the exp activation table while DMAs run (first ACTIVATE of a
    # set pays ~2.7us; do it off the critical path)
    zt = smallp.tile([P, 1], FP32, tag="zt")
    nc.vector.memset(zt, 0.0)
    zo = smallp.tile([P, 1], FP32, tag="zo")
    nc.scalar.activation(zo, zt, AF.Exp)

    # HAM warm-up: real K=128 matmuls that depend only on the gpsimd-built
    # identity (not on any DMA) keep PE busy from t~0 so the 1.2->2.4 GHz
    # un-throttle window elapses before the real work; rhs reads
    # uninitialized qt (junk is fine, the psum result is never read).
    for i in range(10):
        dmy = ps_o.tile([P, SB], FP32, tag="o1", name="dummy")
        nc.tensor.matmul(dmy, lhsT=ident, rhs=qt[:, (i % 8) * SB:(i % 8 + 1) * SB],
                         start=True, stop=True)

    # ---- projections (through the o0/o1 psum regions) -------------------
    def kt_round(g):  # cols [g*1024, (g+1)*1024)
        ps = ps_o.tile([P, TB], FP32, tag="o1", name="ktps")
        for h in range(2):
            nc.tensor.matmul(ps[:, h * SB:(h + 1) * SB], lhsT=wk,
                             rhs=xt[:, g * TB + h * SB:g * TB + (h + 1) * SB],
                             start=True, stop=True)
        nc.scalar.copy(kt[:, g * TB:(g + 1) * TB], ps)

    def qt_round(i, tag):  # cols [i*512, (i+1)*512)
        ps = ps_o.tile([P, SB], FP32, tag=tag, name="qtps")
        nc.tensor.matmul(ps, lhsT=wq, rhs=xt[:, i * SB:(i + 1) * SB],
                         start=True, stop=True)
        nc.vector.tensor_copy(qt[:, i * SB:(i + 1) * SB], ps)

    def va_round(g, on_act):  # s-tiles [g*8, (g+1)*8)
        ps = ps_o.tile([P, 8, P], FP32, tag="o1", name="vaps")
        for j in range(8):
            nc.tensor.matmul(ps[:, j, :], lhsT=xt[:, (g * 8 + j) * P:(g * 8 + j + 1) * P],
                             rhs=wv, start=True, stop=True)
        if on_act:
            nc.scalar.copy(va[:, g * 8:(g + 1) * 8, :], ps)
        else:
            nc.vector.tensor_copy(va[:, g * 8:(g + 1) * 8, :], ps)

    # minimal upfront set: enough for (tb=0, st=0..7)
    kt_round(0)
    qt_round(0, "o0")         # o0 region is free until attn(0,0)
    va_round(0, True)

    # remaining setup rounds, interleaved into the tb=0 main loop
    setup_sched = {
        2: lambda: qt_round(1, "o1"),
        3: lambda: va_round(1, False),
        6: lambda: qt_round(2, "o1"),
        8: lambda: qt_round(3, "o1"),
        9: lambda: va_round(2, False),
        12: lambda: qt_round(4, "o1"),
        14: lambda: qt_round(5, "o1"),
        15: lambda: va_round(3, False),
        18: lambda: qt_round(6, "o1"),
        20: lambda: qt_round(7, "o1"),
        22: lambda: kt_round(1),
    }

    # ---- flash attention main loop --------------------------------------
    out_v = out_d.ap().rearrange("(n p) c -> p n c", p=P)  # [128, 16, 64]
    po = [ps_o.tile([P, TB], FP32, tag=f"o{tb}", name="po") for tb in range(N_TB)]
    ex = [None] * NT

    def scores(tb, st):
        pss = ps_s.tile([P, TB], FP32, tag="pss", name="pss")
        for h in range(2):
            nc.tensor.matmul(pss[:, h * SB:(h + 1) * SB],
                             lhsT=qt[:, st * P:(st + 1) * P],
                             rhs=kt[:, tb * TB + h * SB:tb * TB + (h + 1) * SB],
                             start=True, stop=True)
        e = expp.tile([P, TB], BF16, tag="ex", name="ex")
        if st in DVE_EXP:
            nc.vector.tensor_scalar(e.bitcast(I16), pss, SCHRAUD_A, SCHRAUD_B,
                                    ALU.mult, ALU.add)
        else:
            nc.scalar.activation(e, pss, AF.Exp)
        ex[st] = e

    def attn(tb, st):
        for h in range(2):
            nc.tensor.matmul(po[tb][:, h * SB:(h + 1) * SB],
                             lhsT=va[:, st, :],
                             rhs=ex[st][:, h * SB:(h + 1) * SB],
                             start=(st == 0), stop=(st == NT - 1))

    def finalize(tb):
        # po[c_aug, t] -> transpose 128-chunks back via identity matmuls,
        # normalize by the denominator column, add residual, store.
        osb = osbp.tile([P, TB], BF16, tag="osb")
        nc.scalar.copy(osb[:, 0:SB], po[tb][:, 0:SB])
        nc.vector.tensor_copy(osb[:, SB:TB], po[tb][:, SB:TB])
        # phase-batched (all transposes, all recips, all scales, all adds)
        # so no engine's FIFO head blocks on another engine mid-sequence.
        pts = [ps_o.tile([P, 4, P], FP32, tag=f"o{tb}", name="pt") for _ in range(2)]
        for j in range(8):
            nc.tensor.matmul(pts[j // 4][:, j % 4, :], lhsT=osb[:, j * P:(j + 1) * P],
                             rhs=ident, start=True, stop=True)
        recs = []
        for j in range(8):
            rec = smallp.tile([P, 1], FP32, tag="rec")
            nc.vector.reciprocal(rec, pts[j // 4][:, j % 4, C:C + 1])
            recs.append(rec)
        ots = []
        for j in range(8):
            ot = outp.tile([P, C], FP32, tag="ot")
            nc.scalar.activation(ot, pts[j // 4][:, j % 4, 0:C], AF.Copy,
                                 scale=recs[j])
            ots.append(ot)
        for j in range(8):
            nc.vector.tensor_add(ots[j], ots[j], xr[:, tb * 8 + j, :])
            nc.sync.dma_start(out_v[:, tb * 8 + j, :], ots[j])

    for tb in range(N_TB):
        for st in range(NT):
            if tb == 0 and st in setup_sched:
                setup_sched[st]()
            scores(tb, st)
            if st > 0:
                attn(tb, st - 1)
        attn(tb, NT - 1)
        finalize(tb)


def build():
    nc = bacc.Bacc("TRN2", target_bir_lowering=False, debug=False,
                   num_devices=N_CORES)
    xt_d = nc.dram_tensor("xt", [P, T], BF16, kind="ExternalInput")
    xr_d = nc.dram_tensor("xr", [P, TL // P * C], FP32, kind="ExternalInput")
    wq_d = nc.dram_tensor("wq", [P, P], BF16, kind="ExternalInput")
    wk_d = nc.dram_tensor("wk", [P, P], BF16, kind="ExternalInput")
    wv_d = nc.dram_tensor("wv", [P, P], BF16, kind="ExternalInput")
    out_d = nc.dram_tensor("out", [TL, C], FP32, kind="ExternalOutput")

    with tile.TileContext(nc) as tc, ExitStack() as ctx:
        _emit(tc, ctx, xt_d, xr_d, wq_d, wk_d, wv_d, out_d)
    nc.compile()
    return nc


def make_in_maps(inputs, Wq, bq, Wk, bk, Wv, bv, gamma):
    """Host-side layout prep + sharding into per-core input maps."""
    bf16 = ml_dtypes.bfloat16
    x = np.asarray(inputs, dtype=np.float32).reshape(B, T, C)
    g = float(np.asarray(gamma, np.float32).reshape(-1)[0])

    def w_aug(W, b, scale=1.0):
        w = np.zeros((P, P), dtype=np.float32)
        w[0:C, 0:C] = np.asarray(W, np.float32) * scale
        w[C, 0:C] = np.asarray(b, np.float32) * scale
        return w.astype(bf16)

    wq_np = w_aug(Wq, bq)
    wk_np = w_aug(Wk, bk)
    wv_np = w_aug(Wv, bv, scale=g)      # gamma folded into V
    wv_np[C, C] = bf16(1.0)             # ones column -> softmax denominator

    in_maps = []
    for core in range(N_CORES):
        b_i, h = divmod(core, HALVES)
        xb = x[b_i]
        if h:
            xb = np.concatenate([xb[h * TL:], xb[:h * TL]], axis=0)
        xt_np = np.zeros((P, T), dtype=bf16)
        xt_np[0:C] = xb.T.astype(bf16)
        xt_np[C] = bf16(1.0)
        xr_np = np.ascontiguousarray(
            xb[0:TL].reshape(TL // P, P, C).transpose(1, 0, 2).reshape(P, -1))
        in_maps.append({
            "xt": xt_np, "xr": xr_np,
            "wq": wq_np, "wk": wk_np, "wv": wv_np,
        })
    return in_maps


def assemble(results):
    """Gather per-core [TL, C] outputs into the full [B, 1, T, C]."""
    out = np.empty((B, 1, T, C), dtype=np.float32)
    for core in range(N_CORES):
        b_i, h = divmod(core, HALVES)
        out[b_i, 0, h * TL:(h + 1) * TL, :] = results[core]["out"]
    return out


_NC_CACHE = []


def kernel(inputs, Wq, bq, Wk, bk, Wv, bv, gamma):
    if not _NC_CACHE:
        _NC_CACHE.append(build())
    nc = _NC_CACHE[0]
    in_maps = make_in_maps(inputs, Wq, bq, Wk, bk, Wv, bv, gamma)
    res = run_bass_kernel_spmd(nc, in_maps, list(range(N_CORES)))
    return assemble(res.results)


# revision 12
# speedup vs baseline: 1.4915x; 1.2604x over previous
"""Fused self-attention kernel for Trainium2 (Bass/Tile), SPMD over 8 cores.

Math (per batch b):
    q = x @ Wq + bq ; k = x @ Wk + bk ; v = x @ Wv + bv          [T, C]
    scores[t, s] = k[t] . q[s]      (non-causal, unscaled)
    beta = softmax(scores, axis=s)
    attn[t] = sum_s beta[t, s] * v[s]
    out = gamma * attn + x

Sharding: 8 cores = 4 batches x 2 halves of the output rows t. Each core's
x is rotated so its local 2048 output rows come first (softmax over s is
permutation invariant, so rotating s is safe).

Host-side layout prep (inside kernel(), pure numpy): x is transposed,
ones-augmented and cast to bf16 (xt), the residual slice is pre-tiled
to the SBUF partition layout (xr), and the weights are padded to
[128,128] bf16 with the bias folded in as row 64 (the ones row of xt
multiplies it back in).  gamma is folded into Wv/bv (the ones column of
v_aug that produces the softmax denominator is NOT scaled), so the
device kernel never sees gamma and out = num/denom + x directly.

On-chip: scoresT[s, t] with s on partitions; denominator via the ones
column of v_aug.  No max-subtraction (|scores| < ~60 for normalized
inputs; exp in fp32, overflow at 88).  exp is split between the Scalar
engine (table exp) and the Vector engine (Schraudolph-style bf16 bit
trick: bitcast(round(x*184.665 + 16248.7)) ~= exp(x) to ~3%), since ACT
alone (1 elem/cycle/lane) would be the bottleneck.  The TxT score
matrix never touches HBM.
"""

import numpy as np
import ml_dtypes
from contextlib import ExitStack

import concourse.bass as bass
import concourse.tile as tile
from concourse import bacc, mybir
from concourse.bass_utils import run_bass_kernel_spmd
from concourse.masks import make_identity

FP32 = mybir.dt.float32
BF16 = mybir.dt.bfloat16
I16 = mybir.dt.int16
AF = mybir.ActivationFunctionType
ALU = mybir.AluOpType

B, T, C = 4, 4096, 64
P = 128
HALVES = 2
N_CORES = B * HALVES
TL = T // HALVES      # local output rows per core (2048)
TB = 1024             # t-block width (one PSUM accumulator pair)
N_TB = TL // TB       # 2
SB = 512              # psum-bank-sized matmul free dim
NT = T // P           # 32 s-tiles

# Schraudolph exp in bf16 bits: exp(x) ~= bitcast_bf16(round(A*x + BB))
SCHRAUD_A = 128.0 / np.log(2.0)          # 184.6650
SCHRAUD_B = 16256.0 - 0.0573 * 128.0     # 16248.67 (balanced max rel err ~3%)

# which s-tiles' exp goes to the Vector engine (rest on Scalar engine)
DVE_EXP = set(range(1, NT, 3)) | {30}    # 11 of 32 per t-block


def _emit(tc, ctx, xt_d, xr_d, wq_d, wk_d, wv_d, out_d):
    nc = tc.nc

    const = ctx.enter_context(tc.tile_pool(name="const", bufs=1))
    expp = ctx.enter_context(tc.tile_pool(name="expp", bufs=6))
    osbp = ctx.enter_context(tc.tile_pool(name="osbp", bufs=2))
    outp = ctx.enter_context(tc.tile_pool(name="outp", bufs=8))
    smallp = ctx.enter_context(tc.tile_pool(name="smallp", bufs=8))
    # PSUM (8 banks): 6-deep ring of 1-bank [128,512] tiles (scores run
    # 2-3 s-tiles ahead of attn so exp latency never stalls the PE FIFO),
    # one shared po accumulator region = 2 banks (tb=0 and tb=1 are
    # sequential; tb=1's first matmul just waits for the tb=0 psum->sbuf
    # copy).  Setup rounds and finalize transposes borrow ring slots.
    ps_s = ctx.enter_context(tc.tile_pool(name="ps_s", bufs=6, space="PSUM"))
    ps_o = ctx.enter_context(tc.tile_pool(name="ps_o", bufs=1, space="PSUM"))

    # ---- constants & DMAs ----------------------------------------------
    ident = const.tile([P, P], BF16, tag="ident")
    make_identity(nc, ident)

    wq = const.tile([P, P], BF16, tag="wq")
    wk = const.tile([P, P], BF16, tag="wk")
    wv = const.tile([P, P], BF16, tag="wv")
    xt = const.tile([P, T], BF16, tag="xt")   # rows 0:64 x.T, 64 ones, 65: zeros
    qt = const.tile([P, T], BF16, tag="qt")       # q.T, all s
    kt = const.tile([P, TL], BF16, tag="kt")      # k.T, local t
    va = const.tile([P, NT, P], BF16, tag="va")   # v_aug per s-tile [s,c]

    # DMA order: first xt chunk + weights first (setup starts on them),
    # residual last (only needed at finalize time).
    nc.sync.dma_start(xt[:, 0:TB], xt_d.ap()[:, 0:TB])
    nc.sync.dma_start(wq, wq_d.ap())
    nc.sync.dma_start(wk, wk_d.ap())
    nc.sync.dma_start(wv, wv_d.ap())
    for i in range(1, 4):
        nc.sync.dma_start(xt[:, i * TB:(i + 1) * TB],
                          xt_d.ap()[:, i * TB:(i + 1) * TB])
    xr = const.tile([P, TL // P, C], FP32, tag="xr")  # residual, partition-tiled
    nc.sync.dma_start(xr, xr_d.ap().rearrange("p (n c) -> p n c", c=C))

    # preload the exp activation table while DMAs run (first ACTIVATE of a
    # set pays ~2.7us; do it off the critical path)
    zt = smallp.tile([P, 1], FP32, tag="zt")
    nc.vector.memset(zt, 0.0)
    zo = smallp.tile([P, 1], FP32, tag="zo")
    nc.scalar.activation(zo, zt, AF.Exp)

    # HAM warm-up: real K=128 matmuls that depend only on the gpsimd-built
    # identity (not on any DMA) keep PE busy from t~0 so the 1.2->2.4 GHz
    # un-throttle window elapses before the real work; rhs reads
    # uninitialized qt (junk is fine, the psum result is never read).
    for i in range(10):
        dmy = ps_s.tile([P, SB], FP32, tag="pss", name="dummy")
        nc.tensor.matmul(dmy, lhsT=ident, rhs=qt[:, (i % 8) * SB:(i % 8 + 1) * SB],
                         start=True, stop=True)

    # ---- projections (through 1-bank pss ring slots) --------------------
    def kt_round(g, on_act):  # cols [g*512, (g+1)*512)
        ps = ps_s.tile([P, SB], FP32, tag="pss", name="ktps")
        nc.tensor.matmul(ps, lhsT=wk, rhs=xt[:, g * SB:(g + 1) * SB],
                         start=True, stop=True)
        if on_act:
            nc.scalar.copy(kt[:, g * SB:(g + 1) * SB], ps)
        else:
            nc.vector.tensor_copy(kt[:, g * SB:(g + 1) * SB], ps)

    def qt_round(i):  # cols [i*512, (i+1)*512)
        ps = ps_s.tile([P, SB], FP32, tag="pss", name="qtps")
        nc.tensor.matmul(ps, lhsT=wq, rhs=xt[:, i * SB:(i + 1) * SB],
                         start=True, stop=True)
        nc.vector.tensor_copy(qt[:, i * SB:(i + 1) * SB], ps)

    def va_round(g, on_act):  # s-tiles [g*4, (g+1)*4)
        ps = ps_s.tile([P, 4, P], FP32, tag="pss", name="vaps")
        for j in range(4):
            nc.tensor.matmul(ps[:, j, :], lhsT=xt[:, (g * 4 + j) * P:(g * 4 + j + 1) * P],
                             rhs=wv, start=True, stop=True)
        if on_act:
            nc.scalar.copy(va[:, g * 4:(g + 1) * 4, :], ps)
        else:
            nc.vector.tensor_copy(va[:, g * 4:(g + 1) * 4, :], ps)

    # minimal upfront set: enough for (tb=0, st=0..7)
    kt_round(0, True)
    kt_round(1, True)
    qt_round(0)
    va_round(0, True)
    va_round(1, False)

    # remaining setup rounds, interleaved into the tb=0 main loop
    setup_sched = {
        2: [lambda: qt_round(1)],
        4: [lambda: va_round(2, False)],
        6: [lambda: qt_round(2)],
        8: [lambda: qt_round(3), lambda: va_round(3, False)],
        10: [lambda: va_round(4, False)],
        12: [lambda: qt_round(4)],
        14: [lambda: qt_round(5), lambda: va_round(5, False)],
        16: [lambda: va_round(6, False)],
        18: [lambda: qt_round(6)],
        20: [lambda: qt_round(7), lambda: va_round(7, False)],
        22: [lambda: kt_round(2, False)],
        24: [lambda: kt_round(3, False)],
    }

    # ---- flash attention main loop --------------------------------------
    out_v = out_d.ap().rearrange("(n p) c -> p n c", p=P)  # [128, 16, 64]
    ex = [None] * NT
    po = [None] * N_TB

    def scores(tb, st):
        # two 1-bank half tiles; exp per half right behind its matmul
        e = expp.tile([P, TB], BF16, tag="ex", name="ex")
        for h in range(2):
            pss = ps_s.tile([P, SB], FP32, tag="pss", name="pss")
            nc.tensor.matmul(pss,
                             lhsT=qt[:, st * P:(st + 1) * P],
                             rhs=kt[:, tb * TB + h * SB:tb * TB + (h + 1) * SB],
                             start=True, stop=True)
            eh = e[:, h * SB:(h + 1) * SB]
            if st == NT - 1:
                # endgame critical path: one half on each engine
                on_dve = h == 1
            else:
                on_dve = st in DVE_EXP
            if on_dve:
                nc.vector.tensor_scalar(eh.bitcast(I16), pss, SCHRAUD_A,
                                        SCHRAUD_B, ALU.mult, ALU.add)
            else:
                nc.scalar.activation(eh, pss, AF.Exp)
        ex[st] = e

    def attn(tb, st):
        for h in range(2):
            nc.tensor.matmul(po[tb][:, h * SB:(h + 1) * SB],
                             lhsT=va[:, st, :],
                             rhs=ex[st][:, h * SB:(h + 1) * SB],
                             start=(st == 0), stop=(st == NT - 1))

    def fin_start(tb):
        # free the shared po region ASAP (tb=1's first attn waits on this)
        osb = osbp.tile([P, TB], BF16, tag="osb")
        nc.scalar.copy(osb[:, 0:SB], po[tb][:, 0:SB])
        if tb == N_TB - 1:  # ACT is free after the last exp
            nc.scalar.copy(osb[:, SB:TB], po[tb][:, SB:TB])
        else:
            nc.vector.tensor_copy(osb[:, SB:TB], po[tb][:, SB:TB])
        return osb

    def fin_chunk(tb, osb, half):
        # transpose 4 chunks via identity matmuls, normalize, add residual
        pt = ps_s.tile([P, 4, P], FP32, tag="pss", name="pt")
        for jj in range(4):
            j = half * 4 + jj
            nc.tensor.matmul(pt[:, jj, :], lhsT=osb[:, j * P:(j + 1) * P],
                             rhs=ident, start=True, stop=True)
        recs = []
        for jj in range(4):
            rec = smallp.tile([P, 1], FP32, tag="rec")
            nc.vector.reciprocal(rec, pt[:, jj, C:C + 1])
            recs.append(rec)
        ots = []
        for jj in range(4):
            ot = outp.tile([P, C], FP32, tag="ot")
            nc.scalar.activation(ot, pt[:, jj, 0:C], AF.Copy, scale=recs[jj])
            ots.append(ot)
        for jj in range(4):
            j = half * 4 + jj
            nc.vector.tensor_add(ots[jj], ots[jj], xr[:, tb * 8 + j, :])
            nc.sync.dma_start(out_v[:, tb * 8 + j, :], ots[jj])

    fin_sched = {}
    for tb in range(N_TB):
        po[tb] = ps_o.tile([P, TB], FP32, tag="po", name=f"po{tb}")
        for st in range(NT):
            for f in setup_sched.get(st, []) if tb == 0 else fin_sched.get(st, []):
                f()
            scores(tb, st)
            if st >= 2:
                attn(tb, st - 2)
        attn(tb, NT - 2)
        attn(tb, NT - 1)
        osb = fin_start(tb)
        if tb == 0:
            fin_sched = {3: [lambda: fin_chunk(0, osb, 0)],
                         7: [lambda: fin_chunk(0, osb, 1)]}
        else:
            fin_chunk(tb, osb, 0)
            fin_chunk(tb, osb, 1)


def build():
    nc = bacc.Bacc("TRN2", target_bir_lowering=False, debug=False,
                   num_devices=N_CORES)
    xt_d = nc.dram_tensor("xt", [P, T], BF16, kind="ExternalInput")
    xr_d = nc.dram_tensor("xr", [P, TL // P * C], FP32, kind="ExternalInput")
    wq_d = nc.dram_tensor("wq", [P, P], BF16, kind="ExternalInput")
    wk_d = nc.dram_tensor("wk", [P, P], BF16, kind="ExternalInput")
    wv_d = nc.dram_tensor("wv", [P, P], BF16, kind="ExternalInput")
    out_d = nc.dram_tensor("out", [TL, C], FP32, kind="ExternalOutput")

    with tile.TileContext(nc) as tc, ExitStack() as ctx:
        _emit(tc, ctx, xt_d, xr_d, wq_d, wk_d, wv_d, out_d)
    nc.compile()
    return nc


def make_in_maps(inputs, Wq, bq, Wk, bk, Wv, bv, gamma):
    """Host-side layout prep + sharding into per-core input maps."""
    bf16 = ml_dtypes.bfloat16
    x = np.asarray(inputs, dtype=np.float32).reshape(B, T, C)
    g = float(np.asarray(gamma, np.float32).reshape(-1)[0])

    def w_aug(W, b, scale=1.0):
        w = np.zeros((P, P), dtype=np.float32)
        w[0:C, 0:C] = np.asarray(W, np.float32) * scale
        w[C, 0:C] = np.asarray(b, np.float32) * scale
        return w.astype(bf16)

    wq_np = w_aug(Wq, bq)
    wk_np = w_aug(Wk, bk)
    wv_np = w_aug(Wv, bv, scale=g)      # gamma folded into V
    wv_np[C, C] = bf16(1.0)             # ones column -> softmax denominator

    in_maps = []
    for core in range(N_CORES):
        b_i, h = divmod(core, HALVES)
        xb = x[b_i]
        if h:
            xb = np.concatenate([xb[h * TL:], xb[:h * TL]], axis=0)
        xt_np = np.zeros((P, T), dtype=bf16)
        xt_np[0:C] = xb.T.astype(bf16)
        xt_np[C] = bf16(1.0)
        xr_np = np.ascontiguousarray(
            xb[0:TL].reshape(TL // P, P, C).transpose(1, 0, 2).reshape(P, -1))
        in_maps.append({
            "xt": xt_np, "xr": xr_np,
            "wq": wq_np, "wk": wk_np, "wv": wv_np,
        })
    return in_maps


def assemble(results):
    """Gather per-core [TL, C] outputs into the full [B, 1, T, C]."""
    out = np.empty((B, 1, T, C), dtype=np.float32)
    for core in range(N_CORES):
        b_i, h = divmod(core, HALVES)
        out[b_i, 0, h * TL:(h + 1) * TL, :] = results[core]["out"]
    return out


_NC_CACHE = []


def kernel(inputs, Wq, bq, Wk, bk, Wv, bv, gamma):
    if not _NC_CACHE:
        _NC_CACHE.append(build())
    nc = _NC_CACHE[0]
    in_maps = make_in_maps(inputs, Wq, bq, Wk, bk, Wv, bv, gamma)
    res = run_bass_kernel_spmd(nc, in_maps, list(range(N_CORES)))
    return assemble(res.results)
